# revision 27
# baseline (speedup 1.0000x reference)
"""Trainium2 Bass kernel for nn_BiasWeightLayerPrime.

Computes out[b, n] = x[b, n] * w[n] + v[n] where
    w[n] = sum_p kernel[p, n mod prime_p],  v[n] = sum_p bias[p, n mod prime_p]
over the 168 primes below 1000.

Distribution: the feature axis N = 524288 is sharded across the 8 NeuronCores
(65536 columns each); the batch (64) is kept whole per core.

Layout (v3, transposed + batch-major): columns live on PARTITIONS, batch along
the free axis.  Per core, column c = q*128 + p maps to partition p; within a
tile the free axis is b*NQ + q (batch-major).  w/v are per-(partition, q)
values: a (128, NQ) table slice read with a stride-0 MIDDLE dim broadcasts
each value across the 64 batch elements while the innermost dim stays packed
(step 1) — this keeps the DVE 2x_1p 16-bit fast path engaged (0.52 ns/col)
with no PE broadcast matmul and no PSUM.

Everything is fp16 (tolerance is 2e-2 scale-relative; fp16 contributes
~1e-3): x is converted host-side, the device computes
    y = x * w_rep   (DVE tensor_tensor)
    y = y + v_rep   (DVE tensor_tensor, in place)
and y is written back as fp16, upconverted host-side.  HBM traffic per core
drops from 33.5 MiB (fp32 in/out) to 16.5 MiB.  All transfers use nc.gpsimd
SWDGE (sprays all 16 SDMA engines); the Pool engine is NOT used for compute
(concurrent Pool+DVE tensor ops destructively interfere, measured 4-5x
slowdowns on both).  Tile widths taper at both ends so the first multiply
starts early and the last store drains fast.

Measured (NTFF, core 0): 58-65 us (median ~61) vs the 113.6 us fp32
PE-broadcast baseline; the DMA union timeline is dense at ~336 GB/s ≈ 94%
of the 358 GB/s per-core HBM cap, i.e. the kernel sits at the fp16 memory
roofline (~46 us of bytes + ~8 us fixed preamble + ~3 us drain).
"""

import os

import numpy as np

from concourse import bacc, mybir
import concourse.tile as tile
from concourse.bass_utils import run_bass_kernel_spmd

N_CORES = 8
B = 64
N_FULL = 524288
S = N_FULL // N_CORES        # 65536 columns per core
QTOT = S // 128              # 512 q-groups per core
FREE = QTOT * B              # 32768 free elems per core
# Per-tile free widths (each = NQ_t * 64).  Small edge tiles cut pipeline
# startup/drain latency; big middle tiles keep DMA efficiency high.
WIDTHS = (1024, 2048, 8192, 8192, 8192, 2048, 2048, 1024)
assert sum(WIDTHS) == FREE

_PRIMES = [
    2, 3, 5, 7, 11, 13, 17, 19, 23, 29, 31, 37, 41, 43, 47, 53, 59, 61, 67,
    71, 73, 79, 83, 89, 97, 101, 103, 107, 109, 113, 127, 131, 137, 139, 149,
    151, 157, 163, 167, 173, 179, 181, 191, 193, 197, 199, 211, 223, 227, 229,
    233, 239, 241, 251, 257, 263, 269, 271, 277, 281, 283, 293, 307, 311, 313,
    317, 331, 337, 347, 349, 353, 359, 367, 373, 379, 383, 389, 397, 401, 409,
    419, 421, 431, 433, 439, 443, 449, 457, 461, 463, 467, 479, 487, 491, 499,
    503, 509, 521, 523, 541, 547, 557, 563, 569, 571, 577, 587, 593, 599, 601,
    607, 613, 617, 619, 631, 641, 643, 647, 653, 659, 661, 673, 677, 683, 691,
    701, 709, 719, 727, 733, 739, 743, 751, 757, 761, 769, 773, 787, 797, 809,
    811, 821, 823, 827, 829, 839, 853, 857, 859, 863, 877, 881, 883, 887, 907,
    911, 919, 929, 937, 941, 947, 953, 967, 971, 977, 983, 991, 997,
]


def _prime_mask(table: np.ndarray, n: int) -> np.ndarray:
    """w[j] = sum_p table[p, j mod prime_p] for j in [0, n) — float64 accum."""
    acc = np.zeros(n, dtype=np.float64)
    for i, p in enumerate(_PRIMES):
        row = table[i, :p].astype(np.float64)
        reps = -(-n // p)
        acc += np.tile(row, reps)[:n]
    return acc.astype(np.float32)


def build_bass(widths=WIDTHS):
    """Single-core Bass program over a (128, FREE) fp16 shard."""
    nc = bacc.Bacc("TRN2", target_bir_lowering=False, debug=False)
    f16 = mybir.dt.float16
    x = nc.dram_tensor("x", (128, FREE), f16, kind="ExternalInput")
    # wb[p, q0 + q] = w value for tile-local group q; wb[p, QTOT + ...] = bias
    wb = nc.dram_tensor("wb", (128, 2 * QTOT), f16, kind="ExternalInput")
    out = nc.dram_tensor("out", (128, FREE), f16, kind="ExternalOutput")
    nt = len(widths)

    with tile.TileContext(nc) as tc:
        with (
            tc.tile_pool(name="xp", bufs=nt) as xp,
            tc.tile_pool(name="yp", bufs=min(nt, 4)) as yp,
            tc.tile_pool(name="wbp", bufs=1) as wbp,
        ):
            wbt = wbp.tile([128, 2 * QTOT], f16)

            # Issue the first two x loads before the (small) wb table so
            # data bytes start flowing as early as possible.
            xts = []
            foff = 0
            for w_t in widths:
                xt = xp.tile([128, w_t], f16)
                xts.append((xt, foff, w_t))
                foff += w_t
            # x0 first (first compute gate), then the tiny wb table
            # (also gates the first multiply), then the rest of the x stream.
            for xt, foff, w_t in xts[:1]:
                nc.gpsimd.dma_start(xt[:], x.ap()[:, foff : foff + w_t])
            nc.gpsimd.dma_start(wbt[:], wb.ap())
            for xt, foff, w_t in xts[1:]:
                nc.gpsimd.dma_start(xt[:], x.ap()[:, foff : foff + w_t])

            qoff = 0
            for t, (xt, foff, w_t) in enumerate(xts):
                nq = w_t // B
                yt = yp.tile([128, w_t], f16)
                # batch-major free axis: innermost dim (q, step 1) stays
                # packed so the DVE 2x_1p 16-bit mode engages; the stride-0
                # broadcast of w/b rides the middle (batch) dim.
                x3 = xt[:].rearrange("p (b q) -> p b q", q=nq)
                y3 = yt[:].rearrange("p (b q) -> p b q", q=nq)
                w3 = (
                    wbt[:, qoff : qoff + nq]
                    .unsqueeze(1)
                    .broadcast_to((128, B, nq))
                )
                b3 = (
                    wbt[:, QTOT + qoff : QTOT + qoff + nq]
                    .unsqueeze(1)
                    .broadcast_to((128, B, nq))
                )
                nc.vector.tensor_mul(y3, x3, w3)
                nc.vector.tensor_add(y3, y3, b3)
                nc.gpsimd.dma_start(out.ap()[:, foff : foff + w_t], yt[:])
                qoff += nq

    nc.compile()
    return nc


_NC_CACHE = {}


def _get_nc():
    if "nc" not in _NC_CACHE:
        _NC_CACHE["nc"] = build_bass()
    return _NC_CACHE["nc"]


def _pack_x(x16: np.ndarray) -> np.ndarray:
    """x16 (B, N_FULL) -> (N_CORES, 128, FREE), per-tile batch-major."""
    # (b, c, qtot, p) view of the column axis
    xv = x16.reshape(B, N_CORES, QTOT, 128)
    xt = np.empty((N_CORES, 128, FREE), dtype=np.float16)
    qoff = foff = 0
    for w_t in WIDTHS:
        nq = w_t // B
        blk = xv[:, :, qoff : qoff + nq, :]        # (b, c, nq, p)
        xt[:, :, foff : foff + w_t] = (
            blk.transpose(1, 3, 0, 2).reshape(N_CORES, 128, w_t)
        )
        qoff += nq
        foff += w_t
    return xt


def _unpack_out(ot: np.ndarray) -> np.ndarray:
    """(N_CORES, 128, FREE) fp16 -> (B, N_FULL) fp32."""
    out = np.empty((B, N_CORES, QTOT, 128), dtype=np.float32)
    qoff = foff = 0
    for w_t in WIDTHS:
        nq = w_t // B
        blk = ot[:, :, foff : foff + w_t].reshape(N_CORES, 128, B, nq)
        out[:, :, qoff : qoff + nq, :] = blk.transpose(2, 0, 3, 1)
        qoff += nq
        foff += w_t
    return out.reshape(B, N_FULL)


def kernel(x: np.ndarray, kernel: np.ndarray, bias: np.ndarray) -> np.ndarray:
    x = np.asarray(x, dtype=np.float32)
    ktab = np.asarray(kernel, dtype=np.float32)
    btab = np.asarray(bias, dtype=np.float32)
    assert x.shape == (B, N_FULL), x.shape

    w_full = _prime_mask(ktab, N_FULL)
    v_full = _prime_mask(btab, N_FULL)

    xt = _pack_x(x.astype(np.float16))

    # wb[core, p, q] = w[core*S + q*128 + p]; bias in second half
    wq = w_full.reshape(N_CORES, QTOT, 128).transpose(0, 2, 1)  # (c, p, q)
    vq = v_full.reshape(N_CORES, QTOT, 128).transpose(0, 2, 1)
    wb = np.concatenate([wq, vq], axis=2).astype(np.float16)  # (c, 128, 2*QTOT)

    in_maps = [
        {"x": xt[c], "wb": np.ascontiguousarray(wb[c])} for c in range(N_CORES)
    ]

    nc = _get_nc()
    res = run_bass_kernel_spmd(
        nc,
        in_maps,
        core_ids=list(range(N_CORES)),
        trace=bool(os.environ.get("KERNEL_TRACE")),
    )
    out = _unpack_out(np.stack([r["out"] for r in res.results]))
    if os.environ.get("KERNEL_TRACE"):
        _NC_CACHE["last_exec_time_ns"] = res.exec_time_ns
        _NC_CACHE["last_results"] = res
    return out
